# revision 3
# baseline (speedup 1.0000x reference)
"""Trainium2 Bass kernel for DifferentiableSoftmaxTree NLL (hierarchical
softmax negative log-likelihood).

Math: the 2-way log_softmax at each tree node reduces to a softplus of a
logit difference, so for sample b with path nodes n_k / directions d_k:
    s_k  = features[b] . (node_weights[n_k,:,1] - node_weights[n_k,:,0])
    out[b] = sum_k mask_k * softplus((1-2 d_k) * s_k)

Strategy (data-parallel over batch, 8 cores x 512 samples):
  host precomputes the weight-diff table [49999, 512] f32 (replicated to
  every core) and per-sample path row indices / sign / mask; the device
  kernel, per 128-sample block, gathers the 16 path rows per sample with
  indirect (SWDGE) DMA, multiplies by the sample's feature row on the
  vector engine (one batched tensor_tensor per 4-row chunk, broadcast
  feature operand), reduces each row's product on the scalar (ACT)
  engine via activation accum_out, and applies
  softplus(u) = relu(u) + ln(1+exp(-|u|)) with ACT Abs/Exp/Ln/Relu
  (all in the natural_log_exp_and_others table).
  (tensor_tensor_reduce is avoided: it wedges this runtime.)
"""

import numpy as np
from contextlib import ExitStack

import concourse.bass as bass
import concourse.mybir as mybir
import concourse.tile as tile
from concourse import bass_utils
import concourse.bacc as bacc

NUM_CLASSES = 50000
NUM_INTERNAL = NUM_CLASSES - 1
D = 512
B = 4096
K = 16
N_CORES = 8
BL = B // N_CORES          # samples per core
P = 128                    # partition dim
NBLK = BL // P             # 128-sample blocks per core
GCH = 4                    # path rows gathered per indirect DMA per partition

_AF = mybir.ActivationFunctionType
_OP = mybir.AluOpType


def _build_program():
    nc = bacc.Bacc(
        "TRN2",
        target_bir_lowering=False,
        debug=False,
        enable_asserts=False,
        num_devices=N_CORES,
    )
    feat_ap = nc.dram_tensor("feat", [BL, D], mybir.dt.float32, kind="ExternalInput").ap()
    meta_ap = nc.dram_tensor("meta", [BL, 3 * K], mybir.dt.int32, kind="ExternalInput").ap()
    table_ap = nc.dram_tensor(
        "table", [NUM_INTERNAL, D], mybir.dt.float32, kind="ExternalInput"
    ).ap()
    out_ap = nc.dram_tensor("out", [BL, 1], mybir.dt.float32, kind="ExternalOutput").ap()

    with tile.TileContext(nc) as tc, ExitStack() as ctx:
        feat_pool = ctx.enter_context(tc.tile_pool(name="feat", bufs=2))
        meta_pool = ctx.enter_context(tc.tile_pool(name="meta", bufs=2))
        gath_pool = ctx.enter_context(tc.tile_pool(name="gath", bufs=4))
        prod_pool = ctx.enter_context(tc.tile_pool(name="prod", bufs=3))
        dump_pool = ctx.enter_context(tc.tile_pool(name="dump", bufs=2))
        small_pool = ctx.enter_context(tc.tile_pool(name="small", bufs=2))

        for blk in range(NBLK):
            b0 = blk * P
            feat_t = feat_pool.tile([P, D], mybir.dt.float32, tag="feat")
            nc.sync.dma_start(feat_t[:], feat_ap[b0 : b0 + P, :])
            meta_t = meta_pool.tile([P, 3 * K], mybir.dt.int32, tag="meta")
            nc.sync.dma_start(meta_t[:], meta_ap[b0 : b0 + P, :])

            s_t = small_pool.tile([P, K], mybir.dt.float32, tag="s")
            feat_b = feat_t[:, None, :].to_broadcast([P, GCH, D])
            for c in range(K // GCH):
                g_t = gath_pool.tile([P, GCH * D], mybir.dt.float32, tag="g")
                # HW indirect DMA honours only ONE offset per partition
                # (remaining offset-AP elements are ignored and consecutive
                # rows are streamed instead), so gather one row per
                # instruction into adjacent slices of the chunk tile.
                for j in range(GCH):
                    k = c * GCH + j
                    nc.gpsimd.indirect_dma_start(
                        out=g_t[:, j * D : (j + 1) * D],
                        out_offset=None,
                        in_=table_ap[:],
                        in_offset=bass.IndirectOffsetOnAxis(
                            ap=meta_t[:, k : k + 1], axis=0
                        ),
                    )
                prod_t = prod_pool.tile([P, GCH * D], mybir.dt.float32, tag="p")
                nc.vector.tensor_tensor(
                    out=prod_t[:].rearrange("p (g d) -> p g d", g=GCH),
                    in0=g_t[:].rearrange("p (g d) -> p g d", g=GCH),
                    in1=feat_b,
                    op=_OP.mult,
                )
                dump_t = dump_pool.tile([P, D], mybir.dt.float32, tag="d")
                for j in range(GCH):
                    k = c * GCH + j
                    nc.scalar.activation(
                        dump_t[:],
                        prod_t[:, j * D : (j + 1) * D],
                        _AF.Identity,
                        accum_out=s_t[:, k : k + 1],
                    )

            # u = s * sign ; softplus(u) = relu(u) + ln(1+exp(-|u|))
            u_t = small_pool.tile([P, K], mybir.dt.float32, tag="u")
            nc.vector.tensor_tensor(
                out=u_t[:],
                in0=s_t[:],
                in1=meta_t[:, K : 2 * K].bitcast(mybir.dt.float32),
                op=_OP.mult,
            )
            au_t = small_pool.tile([P, K], mybir.dt.float32, tag="au")
            nc.scalar.activation(au_t[:], u_t[:], _AF.Abs)
            e_t = small_pool.tile([P, K], mybir.dt.float32, tag="e")
            nc.scalar.activation(e_t[:], au_t[:], _AF.Exp, scale=-1.0)
            lp_t = small_pool.tile([P, K], mybir.dt.float32, tag="lp")
            nc.scalar.activation(lp_t[:], e_t[:], _AF.Ln, bias=1.0)
            ru_t = small_pool.tile([P, K], mybir.dt.float32, tag="ru")
            nc.scalar.activation(ru_t[:], u_t[:], _AF.Relu)
            t_t = small_pool.tile([P, K], mybir.dt.float32, tag="t")
            nc.vector.tensor_tensor(
                out=t_t[:], in0=ru_t[:], in1=lp_t[:], op=_OP.add
            )
            tm_t = small_pool.tile([P, K], mybir.dt.float32, tag="tm")
            nc.vector.tensor_tensor(
                out=tm_t[:],
                in0=t_t[:],
                in1=meta_t[:, 2 * K : 3 * K].bitcast(mybir.dt.float32),
                op=_OP.mult,
            )
            res_t = small_pool.tile([P, 1], mybir.dt.float32, tag="res")
            dumpk_t = small_pool.tile([P, K], mybir.dt.float32, tag="dk")
            nc.scalar.activation(
                dumpk_t[:], tm_t[:], _AF.Identity, accum_out=res_t[:]
            )
            nc.sync.dma_start(out_ap[b0 : b0 + P, :], res_t[:])

    nc.compile()
    return nc


_PROGRAM_CACHE = {}


def _get_program():
    if "nc" not in _PROGRAM_CACHE:
        _PROGRAM_CACHE["nc"] = _build_program()
    return _PROGRAM_CACHE["nc"]


def _reset_device():
    # A previously-crashed kernel can leave an exec unit wedged; a
    # client-side axon reset clears it and is near-free otherwise.
    try:
        import ctypes

        lib = ctypes.CDLL("/opt/axon/libaxon_pjrt.so")
        lib.axon_reset.restype = ctypes.c_int64
        lib.axon_reset()
    except Exception:
        pass


def _prepare_inputs(features, targets, node_weights, path_nodes_map, path_directions_map):
    features = np.ascontiguousarray(np.asarray(features, dtype=np.float32))
    targets = np.asarray(targets, dtype=np.int32)
    node_weights = np.asarray(node_weights, dtype=np.float32)
    path_nodes_map = np.asarray(path_nodes_map, dtype=np.int32)
    path_directions_map = np.asarray(path_directions_map, dtype=np.int32)

    table = np.ascontiguousarray(node_weights[:, :, 1] - node_weights[:, :, 0])

    tflat = targets.reshape(-1)
    nodes = path_nodes_map[tflat]            # [B, K] int32
    dirs = path_directions_map[tflat]        # [B, K] int32
    maskb = nodes != -1
    safe = np.where(maskb, nodes, 0).astype(np.int32)
    sgn = np.where(maskb, 1 - 2 * dirs, 1).astype(np.float32)
    maskf = maskb.astype(np.float32)
    meta = np.concatenate(
        [safe, sgn.view(np.int32), maskf.view(np.int32)], axis=1
    )
    meta = np.ascontiguousarray(meta, dtype=np.int32)  # [B, 3K]
    return features, meta, table


def kernel(features, targets, node_weights, path_nodes_map, path_directions_map):
    features, meta, table = _prepare_inputs(
        features, targets, node_weights, path_nodes_map, path_directions_map
    )
    _reset_device()
    nc = _get_program()
    in_maps = [
        {
            "feat": features[i * BL : (i + 1) * BL],
            "meta": meta[i * BL : (i + 1) * BL],
            "table": table,
        }
        for i in range(N_CORES)
    ]
    res = bass_utils.run_bass_kernel_spmd(nc, in_maps, core_ids=list(range(N_CORES)))
    out = np.concatenate([res.results[i]["out"].reshape(-1) for i in range(N_CORES)])
    return out.astype(np.float32)


# revision 4
# speedup vs baseline: 1.1423x; 1.1423x over previous
"""Trainium2 Bass kernel for DifferentiableSoftmaxTree NLL (hierarchical
softmax negative log-likelihood).

Math: the 2-way log_softmax at each tree node reduces to a softplus of a
logit difference, so for sample b with path nodes n_k / directions d_k:
    s_k  = features[b] . (node_weights[n_k,:,1] - node_weights[n_k,:,0])
    out[b] = sum_k mask_k * softplus((1-2 d_k) * s_k)

Strategy (data-parallel over batch, 8 cores x 512 samples):
  The host materialises a per-class PATH table
      ptab[c] = concat_k( node_weights[n_k(c),:,1] - node_weights[n_k(c),:,0] )
  of shape [50000, 16*512] f32 (masked levels zeroed), replicated to every
  core. Each sample's entire root->leaf path is then ONE contiguous 32KB
  row, gathered with a single-offset indirect (SWDGE) DMA per 128-sample
  block -- 4 gather instructions per core instead of 64, amortising the
  Q7 descriptor-emission cost that otherwise dominates (measured 123us of
  160us with per-level gathers).
  Compute per block: one batched in-place tensor_tensor multiply against
  the broadcast feature row (DVE), per-level reductions split between DVE
  (one batched 3-level tensor_reduce) and ACT (13 activation accum_out
  ops) to balance the two engines, then
  softplus(u) = relu(u) + ln(1+exp(-|u|)) on ACT (Abs/Exp/Ln/Relu all live
  in the natural_log_exp_and_others table) and a masked sum.
  (tensor_tensor_reduce is avoided: it wedges this runtime. Multi-offset
  indirect DMA is avoided: HW honours only the first offset per
  partition.)
"""

import numpy as np
from contextlib import ExitStack

import concourse.bass as bass
import concourse.mybir as mybir
import concourse.tile as tile
from concourse import bass_utils
import concourse.bacc as bacc

NUM_CLASSES = 50000
NUM_INTERNAL = NUM_CLASSES - 1
D = 512
B = 4096
K = 16
N_CORES = 8
BL = B // N_CORES          # samples per core
P = 128                    # partition dim
NBLK = BL // P             # 128-sample blocks per core
RDVE = 3                   # levels reduced on DVE (rest on ACT)

_AF = mybir.ActivationFunctionType
_OP = mybir.AluOpType


def _build_program():
    nc = bacc.Bacc(
        "TRN2",
        target_bir_lowering=False,
        debug=False,
        enable_asserts=False,
        num_devices=N_CORES,
    )
    feat_ap = nc.dram_tensor("feat", [BL, D], mybir.dt.float32, kind="ExternalInput").ap()
    meta_ap = nc.dram_tensor("meta", [BL, 1 + 2 * K], mybir.dt.int32, kind="ExternalInput").ap()
    ptab_ap = nc.dram_tensor(
        "ptab", [NUM_CLASSES, K * D], mybir.dt.float32, kind="ExternalInput"
    ).ap()
    out_ap = nc.dram_tensor("out", [BL, 1], mybir.dt.float32, kind="ExternalOutput").ap()

    with tile.TileContext(nc) as tc, ExitStack() as ctx:
        feat_pool = ctx.enter_context(tc.tile_pool(name="feat", bufs=2))
        meta_pool = ctx.enter_context(tc.tile_pool(name="meta", bufs=2))
        gath_pool = ctx.enter_context(tc.tile_pool(name="gath", bufs=3))
        dump_pool = ctx.enter_context(tc.tile_pool(name="dump", bufs=2))
        small_pool = ctx.enter_context(tc.tile_pool(name="small", bufs=2))

        for blk in range(NBLK):
            b0 = blk * P
            feat_t = feat_pool.tile([P, D], mybir.dt.float32, tag="feat")
            nc.sync.dma_start(feat_t[:], feat_ap[b0 : b0 + P, :])
            meta_t = meta_pool.tile([P, 1 + 2 * K], mybir.dt.int32, tag="meta")
            nc.sync.dma_start(meta_t[:], meta_ap[b0 : b0 + P, :])

            # one 32KB-per-sample gather of the whole path
            g_t = gath_pool.tile([P, K * D], mybir.dt.float32, tag="g")
            nc.gpsimd.indirect_dma_start(
                out=g_t[:],
                out_offset=None,
                in_=ptab_ap[:],
                in_offset=bass.IndirectOffsetOnAxis(ap=meta_t[:, 0:1], axis=0),
            )

            # in-place batched multiply by the sample's feature row
            g3 = g_t[:].rearrange("p (k d) -> p k d", k=K)
            nc.vector.tensor_tensor(
                out=g3,
                in0=g3,
                in1=feat_t[:, None, :].to_broadcast([P, K, D]),
                op=_OP.mult,
            )

            # per-level reduction: s[:, k] = sum_d g3[:, k, :]
            s_t = small_pool.tile([P, K], mybir.dt.float32, tag="s")
            nc.vector.tensor_reduce(
                out=s_t[:, 0:RDVE],
                in_=g_t[:, 0 : RDVE * D].rearrange("p (k d) -> p k d", k=RDVE),
                axis=mybir.AxisListType.X,
                op=_OP.add,
            )
            dump_t = dump_pool.tile([P, D], mybir.dt.float32, tag="d")
            for k in range(RDVE, K):
                nc.scalar.activation(
                    dump_t[:],
                    g_t[:, k * D : (k + 1) * D],
                    _AF.Identity,
                    accum_out=s_t[:, k : k + 1],
                )

            # u = s * sign ; softplus(u) = relu(u) + ln(1+exp(-|u|))
            u_t = small_pool.tile([P, K], mybir.dt.float32, tag="u")
            nc.vector.tensor_tensor(
                out=u_t[:],
                in0=s_t[:],
                in1=meta_t[:, 1 : 1 + K].bitcast(mybir.dt.float32),
                op=_OP.mult,
            )
            au_t = small_pool.tile([P, K], mybir.dt.float32, tag="au")
            nc.scalar.activation(au_t[:], u_t[:], _AF.Abs)
            e_t = small_pool.tile([P, K], mybir.dt.float32, tag="e")
            nc.scalar.activation(e_t[:], au_t[:], _AF.Exp, scale=-1.0)
            lp_t = small_pool.tile([P, K], mybir.dt.float32, tag="lp")
            nc.scalar.activation(lp_t[:], e_t[:], _AF.Ln, bias=1.0)
            ru_t = small_pool.tile([P, K], mybir.dt.float32, tag="ru")
            nc.scalar.activation(ru_t[:], u_t[:], _AF.Relu)
            t_t = small_pool.tile([P, K], mybir.dt.float32, tag="t")
            nc.vector.tensor_tensor(
                out=t_t[:], in0=ru_t[:], in1=lp_t[:], op=_OP.add
            )
            tm_t = small_pool.tile([P, K], mybir.dt.float32, tag="tm")
            nc.vector.tensor_tensor(
                out=tm_t[:],
                in0=t_t[:],
                in1=meta_t[:, 1 + K : 1 + 2 * K].bitcast(mybir.dt.float32),
                op=_OP.mult,
            )
            res_t = small_pool.tile([P, 1], mybir.dt.float32, tag="res")
            dumpk_t = small_pool.tile([P, K], mybir.dt.float32, tag="dk")
            nc.scalar.activation(
                dumpk_t[:], tm_t[:], _AF.Identity, accum_out=res_t[:]
            )
            nc.sync.dma_start(out_ap[b0 : b0 + P, :], res_t[:])

    nc.compile()
    return nc


_PROGRAM_CACHE = {}


def _get_program():
    if "nc" not in _PROGRAM_CACHE:
        _PROGRAM_CACHE["nc"] = _build_program()
    return _PROGRAM_CACHE["nc"]


def _reset_device():
    # A previously-crashed kernel can leave an exec unit wedged; a
    # client-side axon reset clears it and is near-free otherwise.
    try:
        import ctypes

        lib = ctypes.CDLL("/opt/axon/libaxon_pjrt.so")
        lib.axon_reset.restype = ctypes.c_int64
        lib.axon_reset()
    except Exception:
        pass


def _prepare_inputs(features, targets, node_weights, path_nodes_map, path_directions_map):
    features = np.ascontiguousarray(np.asarray(features, dtype=np.float32))
    targets = np.asarray(targets, dtype=np.int32)
    node_weights = np.asarray(node_weights, dtype=np.float32)
    path_nodes_map = np.asarray(path_nodes_map, dtype=np.int32)
    path_directions_map = np.asarray(path_directions_map, dtype=np.int32)

    diff = node_weights[:, :, 1] - node_weights[:, :, 0]     # [N_INT, D]
    maskmap = (path_nodes_map != -1)                          # [C, K]
    safemap = np.where(maskmap, path_nodes_map, 0)
    # per-class contiguous path table, masked levels zeroed: [C, K*D]
    ptab = diff[safemap]                                      # [C, K, D]
    ptab *= maskmap[:, :, None]
    ptab = np.ascontiguousarray(ptab.reshape(NUM_CLASSES, K * D), dtype=np.float32)

    tflat = targets.reshape(-1)
    dirs = path_directions_map[tflat]                         # [B, K]
    maskb = maskmap[tflat]
    sgn = np.where(maskb, 1 - 2 * dirs, 1).astype(np.float32)
    maskf = maskb.astype(np.float32)
    meta = np.concatenate(
        [tflat[:, None], sgn.view(np.int32), maskf.view(np.int32)], axis=1
    )
    meta = np.ascontiguousarray(meta, dtype=np.int32)         # [B, 1+2K]
    return features, meta, ptab


def kernel(features, targets, node_weights, path_nodes_map, path_directions_map):
    features, meta, ptab = _prepare_inputs(
        features, targets, node_weights, path_nodes_map, path_directions_map
    )
    _reset_device()
    nc = _get_program()
    in_maps = [
        {
            "feat": features[i * BL : (i + 1) * BL],
            "meta": meta[i * BL : (i + 1) * BL],
            "ptab": ptab,
        }
        for i in range(N_CORES)
    ]
    res = bass_utils.run_bass_kernel_spmd(nc, in_maps, core_ids=list(range(N_CORES)))
    out = np.concatenate([res.results[i]["out"].reshape(-1) for i in range(N_CORES)])
    return out.astype(np.float32)
